# revision 1
# baseline (speedup 1.0000x reference)
"""Trainium2 Bass kernel for nn_AdjacencyGenerator (GNN message passing).

Strategy: edges are grouped by dst node and packed into 8 cores x 128
partitions x F_CAP slots (all edges of a dst node land contiguously in one
partition row). The segment softmax then becomes 4 segmented scans
(tensor_tensor_scan) on a [128, F_CAP] scalar plane -- no collectives needed.
Per-edge [E, D] compute is feature-major (channels on partitions) so matmuls
chain without transposes; per-node K/V/Q tables are built once per core and
gathered per edge with dma_gather(transpose=True). All matmuls run in bf16.
"""
import numpy as np
import ml_dtypes

import concourse.bass as bass
import concourse.bacc as bacc
import concourse.tile as tile
from concourse import mybir
from concourse.bass_utils import run_bass_kernel_spmd
from concourse.masks import make_identity


bf16 = ml_dtypes.bfloat16
F32 = mybir.dt.float32
BF = mybir.dt.bfloat16
I16 = mybir.dt.int16

P = 128
D = 256
DB = 2            # D / 128
H = 768
HB = 6            # H / 128
N_CORES = 8
F_CAP = 160       # slots per partition
F_BLK = 16        # slot columns per chunk
NCH = F_CAP // F_BLK
EC = P * F_BLK    # edges per chunk = 2048
GE = 512          # edges per group
NGRP = EC // GE   # 4
GF = GE // P      # 4 plane columns per group
E_CAP = P * F_CAP
NODES = 10000
NODES_P = 10112   # 79 * 128
NG = NODES_P // P
EPS = 1e-5
NEG = -1e30

USE_ACT_LRELU = False  # ACT Lrelu gives wrong results on HW; use stt max(0.2x, x)


# ----------------------------------------------------------------------------
# host-side packing
# ----------------------------------------------------------------------------

def pack(edge_index, n_nodes):
    import heapq
    src, dst = edge_index[0], edge_index[1]
    E = src.shape[0]
    deg = np.bincount(dst, minlength=n_nodes)
    order = np.argsort(dst, kind="stable")
    starts = np.zeros(n_nodes + 1, np.int64)
    np.cumsum(deg, out=starts[1:])
    n_bins = N_CORES * P
    node_order = np.argsort(-deg, kind="stable")
    bins = [[] for _ in range(n_bins)]
    heap = [(-F_CAP, b) for b in range(n_bins)]
    heapq.heapify(heap)
    for n in node_order:
        d = int(deg[n])
        negrem, b = heapq.heappop(heap)
        rem = -negrem
        if rem < d:
            raise RuntimeError(f"packing failed: deg {d} rem {rem}")
        bins[b].append(n)
        heapq.heappush(heap, (-(rem - d), b))
    src_idx = np.zeros((N_CORES, P, F_CAP), np.int32)
    dst_idx = np.zeros((N_CORES, P, F_CAP), np.int32)
    m_cont = np.zeros((N_CORES, P, F_CAP), np.float32)
    is_last = np.ones((N_CORES, P, F_CAP), np.float32)
    orig = np.full((N_CORES, P, F_CAP), -1, np.int64)
    for b in range(n_bins):
        c, p = b // P, b % P
        f = 0
        for n in bins[b]:
            d = int(deg[n])
            eids = order[starts[n]:starts[n] + d]
            src_idx[c, p, f:f + d] = src[eids]
            dst_idx[c, p, f:f + d] = n
            m_cont[c, p, f + 1:f + d] = 1.0
            is_last[c, p, f:f + d - 1] = 0.0
            orig[c, p, f:f + d] = eids
            f += d
    assert (orig >= 0).sum() == E
    return src_idx, dst_idx, m_cont, is_last, orig


def gather_idx_layout(slot_idx):
    """[P, F_CAP] int32 slot indices -> dma_gather idx tile [P, F_CAP*8] int16.

    Gather order within chunk c: i = p + 128*j  (slot (p, c*F_BLK+j)).
    idx position i -> partition i%16, column i//16; replicated to all 8 groups.
    """
    out = np.zeros((P, NCH * EC // 16), np.int16)
    for c in range(NCH):
        vals = slot_idx[:, c * F_BLK:(c + 1) * F_BLK]  # [P, F_BLK]
        flat = vals.T.reshape(-1)  # flat[i] = vals[i % 128 ... ] with i = p + 128*j
        cols = flat.reshape(EC // 16, 16).T  # [16, 128]
        out[:16, c * (EC // 16):(c + 1) * (EC // 16)] = cols
    for g in range(1, 8):
        out[g * 16:(g + 1) * 16] = out[:16]
    return out


def feature_blocks(v):
    """[dout] vector -> [128, dout//128] feature-major (partition = ch%128)."""
    d = v.shape[0]
    return np.ascontiguousarray(v.reshape(d // P, P).T).astype(np.float32)


# ----------------------------------------------------------------------------
# device graph
# ----------------------------------------------------------------------------

def build_nc():
    nc = bacc.Bacc("TRN2", target_bir_lowering=False, debug=False,
                   num_devices=N_CORES)

    dp = lambda n, s, d: nc.declare_dram_parameter(n, list(s), d, isOutput=False).ap()
    x_d = dp("x_pad", [NODES_P, D], F32)
    wspec = {"wtab": (D, 5 * D), "wq1t": (D, D), "wff0t": (D, D),
             "wff1t": (D, D), "w3t": (D, H), "w4t": (H, H), "w5t": (H, D),
             "wvt": (D, 1)}
    wd = {n: dp(n, list(s), F32) for n, s in wspec.items()}
    bq1_d = dp("bq1_t", [P, DB], F32)
    bff0_d = dp("bff0_t", [P, DB], F32)
    bff1_d = dp("bff1_t", [P, DB], F32)
    b3_d = dp("b3_t", [P, HB], F32)
    b4_d = dp("b4_t", [P, HB], F32)
    b5_d = dp("b5_t", [P, DB], F32)
    gf_d = dp("gf_t", [P, DB], F32)
    bfh_d = dp("bf_t", [P, DB], F32)
    bvec_d = dp("bvec_t", [1, 1], F32)
    btab_d = dp("btab_r", [P, 5 * D], F32)
    wsff0_d = dp("wsff0", [1, D], F32)
    wsff1_d = dp("wsff1", [1, D], F32)
    wsv_d = dp("wsv", [1, 1], F32)
    gsrc_d = dp("gsrc", [P, NCH * EC // 16], I16)
    gdst_d = dp("gdst", [P, NCH * EC // 16], I16)
    stadd_d = dp("st_add", [P, F_CAP], F32)
    mcont_d = dp("m_cont", [P, F_CAP], F32)
    nlast_d = dp("notlast", [P, F_CAP], F32)
    islast_d = dp("is_last", [P, F_CAP], F32)

    xspk_d = dp("xspk", [NCH, P, DB, EC], BF)
    xdpk_d = dp("xdpk", [NCH, P, DB, EC], BF)
    bq0_d = dp("bq0_t", [P, DB], F32)
    bk0_d = dp("bk0_t", [P, DB], F32)
    bv0_d = dp("bv0_t", [P, DB], F32)
    bk1_d = dp("bk1_t", [P, DB], F32)
    bv1_d = dp("bv1_t", [P, DB], F32)

    out_d = nc.declare_dram_parameter("out", [E_CAP], F32, isOutput=True).ap()

    tab_d = nc.dram_tensor("tab", [NODES_P, 5 * D], BF).ap()
    TQ0, TK0, TV0, TK1, TV1 = 0, D, 2 * D, 3 * D, 4 * D
    q1s_d = nc.dram_tensor("q1s", [NCH, P, DB, EC], BF).ap()
    q0s_d = nc.dram_tensor("q0s", [NCH, P, DB, EC], BF).ap()

    AT = mybir.ActivationFunctionType
    OP = mybir.AluOpType

    with tile.TileContext(nc) as tc:
        _cms = []
        def open_pool(**kw):
            cm = tc.tile_pool(**kw)
            _cms.append(cm)
            return cm.__enter__()
        cpool = open_pool(name="const", bufs=1)

        # ---- persistent constants -------------------------------------------
        ident = cpool.tile([P, P], F32, tag="ident")
        make_identity(nc, ident[:])
        ones_col = cpool.tile([P, 1], BF, tag="ones_col")
        nc.vector.memset(ones_col[:], 1.0)
        ones_row = cpool.tile([1, P], BF, tag="ones_row")
        nc.vector.memset(ones_row[:], 1.0)
        zero_c = cpool.tile([P, 1], F32, tag="zero_c")
        nc.vector.memset(zero_c[:], 0.0)
        eps_c = cpool.tile([P, 1], F32, tag="eps_c")
        nc.vector.memset(eps_c[:], EPS)

        wstage_cm = tc.tile_pool(name="wstage", bufs=2)
        wstage = wstage_cm.__enter__()

        def load_w(name):
            din, dout = wspec[name]
            dinb = din // P
            t32 = wstage.tile([P, dinb, dout], F32, tag="wstg")
            t = cpool.tile([P, dinb, dout], BF, tag=name)
            for i in range(dinb):
                nc.sync.dma_start(t32[:, i, :], wd[name][i * P:(i + 1) * P, :])
            nc.vector.tensor_copy(t[:], t32[:])
            return t

        w_sb = {n: load_w(n) for n in wspec}

        def load_f32(name, ap, shape):
            t = cpool.tile(list(shape), F32, tag=name)
            nc.sync.dma_start(t[:], ap)
            return t

        bq1 = load_f32("bq1", bq1_d, [P, DB])
        bq0c = load_f32("bq0c", bq0_d, [P, DB])
        bk0c = load_f32("bk0c", bk0_d, [P, DB])
        bv0c = load_f32("bv0c", bv0_d, [P, DB])
        bk1c = load_f32("bk1c", bk1_d, [P, DB])
        bv1c = load_f32("bv1c", bv1_d, [P, DB])
        bff0 = load_f32("bff0", bff0_d, [P, DB])
        bff1 = load_f32("bff1", bff1_d, [P, DB])
        b3 = load_f32("b3", b3_d, [P, HB])
        b4 = load_f32("b4", b4_d, [P, HB])
        b5 = load_f32("b5", b5_d, [P, DB])
        gf = load_f32("gf", gf_d, [P, DB])
        bfh = load_f32("bfh", bfh_d, [P, DB])
        bvec = load_f32("bvec", bvec_d, [1, 1])
        btab = load_f32("btab", btab_d, [P, 5 * D])

        def load_ws(name, ap, dout):
            t32 = wstage.tile([1, dout], F32, tag="wsstg")
            t = cpool.tile([1, dout], BF, tag=name)
            nc.sync.dma_start(t32[:, :dout], ap)
            nc.vector.tensor_copy(t[:], t32[:, :dout])
            return t
        wsff0 = load_ws("wsff0", wsff0_d, D)
        wsff1 = load_ws("wsff1", wsff1_d, D)
        wsv = load_ws("wsv", wsv_d, 1)
        wstage_cm.__exit__(None, None, None)

        gsrc = cpool.tile([P, NCH * EC // 16], I16, tag="gsrci")
        nc.sync.dma_start(gsrc[:], gsrc_d)
        gdst = cpool.tile([P, NCH * EC // 16], I16, tag="gdsti")
        nc.sync.dma_start(gdst[:], gdst_d)
        st_add = load_f32("st_add", stadd_d, [P, F_CAP])
        m_cont = load_f32("m_cont", mcont_d, [P, F_CAP])
        notlast = load_f32("notlast", nlast_d, [P, F_CAP])
        is_last = load_f32("is_last", islast_d, [P, F_CAP])

        alpha = cpool.tile([P, F_CAP], F32, tag="alpha")
        attn = cpool.tile([P, F_CAP], F32, tag="attn")
        sc1 = cpool.tile([P, F_CAP], F32, tag="sc1")
        sc2 = cpool.tile([P, F_CAP], F32, tag="sc2")
        sc3 = cpool.tile([P, F_CAP], F32, tag="sc3")


        # ---- pools for the edge pipeline ------------------------------------
        gpool = open_pool(name="gath", bufs=2)
        kpool = open_pool(name="keep", bufs=2)
        wpool = open_pool(name="work", bufs=2)
        ps_mm = open_pool(name="ps_mm", bufs=3, space="PSUM")
        ps_st = open_pool(name="ps_st", bufs=1, space="PSUM")
        ps_bc = open_pool(name="ps_bc", bufs=3, space="PSUM")
        ps_tr = open_pool(name="ps_tr", bufs=1, space="PSUM")
        rpool = open_pool(name="rowp", bufs=1)
        g3pool = open_pool(name="g3p", bufs=1)
        hbpool = open_pool(name="hbp", bufs=1)

        nidx_reg = nc.gpsimd.to_reg(EC)
        _gq = [0]

        def gather(col_off, idx_tile, c, tag, pool=None):
            t = (pool or gpool).tile([P, DB, EC], BF, tag=tag)
            nc.gpsimd.dma_gather(
                out_ap=t[:], in_ap=tab_d[:, col_off:col_off + D],
                idxs_ap=idx_tile[:, c * (EC // 16):(c + 1) * (EC // 16)],
                num_idxs=EC, num_idxs_reg=nidx_reg, elem_size=D,
                elem_step=5 * D, transpose=True, single_packet=False,
                queue_num=0)
            return t

        def edge_stats(data_ap2, ps_cols, g):
            """data slice [P, DB, GE] bf16 -> psum plane cols [P, GF]."""
            for t in range(GF):
                for i in range(DB):
                    nc.tensor.matmul(
                        ps_cols[:, g * GF + t:g * GF + t + 1],
                        lhsT=data_ap2[:, i, t * P:(t + 1) * P],
                        rhs=ones_col[:], start=(i == 0), stop=(i == DB - 1))

        def bcast_rows(plane_ap, cols, tag):
            """plane [P, cols] f32 -> bf16 row [1, cols*128] in slot order."""
            ps_t = ps_tr.tile([cols, P], F32, tag="trps")
            nc.tensor.transpose(ps_t[:], plane_ap, ident[:])
            sT = wpool.tile([cols, P], BF, tag="sT")
            nc.vector.tensor_copy(sT[:], ps_t[:])
            row = rpool.tile([1, cols * P], BF, tag="rowbc")
            nc.scalar.dma_start(row[:], sT[:])
            return row

        def k1_bcast(row_ap, g, tag):
            """row slice [1, GE] -> psum [128, GE] replicated."""
            ps = ps_bc.tile([P, GE], F32, tag=tag)
            nc.tensor.matmul(ps[:], lhsT=ones_row[:],
                             rhs=row_ap[:, g * GE:(g + 1) * GE],
                             start=True, stop=True)
            return ps

        def ln_ab(sum_ps, sq_ps, tag):
            """stats psums [P, F_BLK] -> (a|b) = (rstd | -mean*rstd) [P, 2*F_BLK]."""
            ab = wpool.tile([P, 2 * F_BLK], F32, tag="ab")
            mean = wpool.tile([P, F_BLK], F32, tag="lnm")
            var = wpool.tile([P, F_BLK], F32, tag="lnv")
            m2 = wpool.tile([P, F_BLK], F32, tag="lnm2")
            nc.vector.tensor_scalar(mean[:], sum_ps[:], 1.0 / D, None, op0=OP.mult)
            nc.vector.tensor_scalar(var[:], sq_ps[:], 1.0 / D, None, op0=OP.mult)
            nc.vector.tensor_tensor(m2[:], mean[:], mean[:], op=OP.mult)
            nc.vector.tensor_tensor(var[:], var[:], m2[:], op=OP.subtract)
            a = ab[:, 0:F_BLK]
            b = ab[:, F_BLK:2 * F_BLK]
            nc.scalar.activation(a, var[:], AT.Sqrt, bias=eps_c[:])
            nc.vector.reciprocal(a, a)
            nc.vector.scalar_tensor_tensor(b, mean[:], -1.0, a,
                                           op0=OP.mult, op1=OP.mult)
            return ab

        def scans():
            nc.vector.tensor_tensor_scan(sc1[:], st_add[:], alpha[:], NEG,
                                         op0=OP.add, op1=OP.max)
            nc.vector.tensor_tensor(sc1[:], sc1[:], is_last[:], op=OP.mult)
            nc.vector.tensor_tensor_scan(sc2[:, ::-1], notlast[:, ::-1],
                                         sc1[:, ::-1], 0.0, op0=OP.mult, op1=OP.add)
            nc.vector.tensor_tensor(sc2[:], alpha[:], sc2[:], op=OP.subtract)
            nc.scalar.activation(sc2[:], sc2[:], AT.Exp, bias=zero_c[:])
            nc.vector.tensor_tensor_scan(sc1[:], m_cont[:], sc2[:], 0.0,
                                         op0=OP.mult, op1=OP.add)
            nc.vector.tensor_tensor(sc1[:], sc1[:], is_last[:], op=OP.mult)
            nc.vector.tensor_tensor_scan(sc3[:, ::-1], notlast[:, ::-1],
                                         sc1[:, ::-1], 0.0, op0=OP.mult, op1=OP.add)
            nc.vector.reciprocal(sc3[:], sc3[:])
            nc.vector.tensor_tensor(attn[:], sc2[:], sc3[:], op=OP.mult)

        def mm_group(rhs_fn, w, dinb, doutb, tag, bias=None, act=None,
                     brow_g=None, wsum=None, pool=None):
            """One group's matmul: out [P, doutb, GE] = act(w.T @ rhs + bias)."""
            out = (pool or wpool).tile([P, doutb, GE], BF, tag=tag)
            for o in range(doutb):
                ps = ps_mm.tile([P, GE], F32, tag="mmps")
                for i in range(dinb):
                    nc.tensor.matmul(ps[:], lhsT=w[:, i, o * P:(o + 1) * P],
                                     rhs=rhs_fn(i),
                                     start=(i == 0),
                                     stop=(i == dinb - 1 and wsum is None))
                if wsum is not None:
                    nc.tensor.matmul(ps[:], lhsT=wsum[:, o * P:(o + 1) * P],
                                     rhs=brow_g, start=False, stop=True)
                dst = out[:, o, :]
                if act == "lrelu" and USE_ACT_LRELU:
                    nc.scalar.activation(dst, ps[:], AT.Lrelu,
                                         bias=bias[:, o:o + 1], alpha=0.2)
                elif act == "lrelu":
                    nc.scalar.activation(dst, ps[:], AT.Identity,
                                         bias=bias[:, o:o + 1])
                    nc.vector.scalar_tensor_tensor(dst, dst, 0.2, dst,
                                                   op0=OP.mult, op1=OP.max)
                elif bias is not None:
                    nc.scalar.activation(dst, ps[:], AT.Identity,
                                         bias=bias[:, o:o + 1])
                else:
                    nc.vector.tensor_copy(dst, ps[:])
            return out

        gsl = lambda t, i, g: t[:, i, g * GE:(g + 1) * GE]

        # ===== phase A0: alpha0 (host-packed x + on-device projection) ======
        def proj_a0(xg, toff, bias, out):
            for g in range(NGRP):
                for o in range(DB):
                    ps = ps_mm.tile([P, GE], F32, tag="mmps")
                    for i in range(DB):
                        nc.tensor.matmul(
                            ps[:], lhsT=w_sb["wtab"][:, i, toff + o * P:toff + (o + 1) * P],
                            rhs=xg[:, i, g * GE:(g + 1) * GE],
                            start=(i == 0), stop=(i == DB - 1))
                    nc.scalar.activation(out[:, o, g * GE:(g + 1) * GE], ps[:],
                                         AT.Identity, bias=bias[:, o:o + 1])

        def proj_g(xg, g, toff, bias, tag):
            vp = wpool.tile([P, DB, GE], BF, tag=tag)
            for o in range(DB):
                ps = ps_mm.tile([P, GE], F32, tag="mmps")
                for i in range(DB):
                    nc.tensor.matmul(
                        ps[:], lhsT=w_sb["wtab"][:, i, toff + o * P:toff + (o + 1) * P],
                        rhs=xg[:, i, g * GE:(g + 1) * GE],
                        start=(i == 0), stop=(i == DB - 1))
                nc.scalar.activation(vp[:, o, :], ps[:], AT.Identity,
                                     bias=bias[:, o:o + 1])
            return vp

        for c in range(NCH):
            xd = kpool.tile([P, DB, EC], BF, tag="epi", name=f"xd{c}")
            nc.sync.dma_start(xd[:], xdpk_d[c])
            xs = kpool.tile([P, DB, EC], BF, tag="t", name=f"xs{c}")
            nc.sync.dma_start(xs[:], xspk_d[c])
            q0 = gpool.tile([P, DB, EC], BF, tag="g1", name=f"q0a{c}")
            proj_a0(xd, TQ0, bq0c, q0)
            nc.sync.dma_start(q0s_d[c], q0[:])
            k0 = gpool.tile([P, DB, EC], BF, tag="g2", name=f"k0a{c}")
            proj_a0(xs, TK0, bk0c, k0)
            ps_a = ps_st.tile([P, F_BLK], F32, tag="sums")
            for g in range(NGRP):
                prod = wpool.tile([P, DB, GE], BF, tag="prod")
                nc.vector.tensor_tensor(
                    prod[:], q0[:, :, g * GE:(g + 1) * GE],
                    k0[:, :, g * GE:(g + 1) * GE], op=OP.mult)
                edge_stats(prod[:], ps_a[:], g)
            nc.vector.tensor_copy(alpha[:, c * F_BLK:(c + 1) * F_BLK], ps_a[:])

        scans()

        # ===== pass C0 + A1 (1-chunk software pipeline) ======================
        c0state = {}

        def c0_front(c):
            arow = bcast_rows(attn[:, c * F_BLK:(c + 1) * F_BLK], F_BLK, "ar")
            q0 = gpool.tile([P, DB, EC], BF, tag="g1", name=f"q0l{c}")
            nc.sync.dma_start(q0[:], q0s_d[c])
            xs = kpool.tile([P, DB, EC], BF, tag="t", name=f"xsc0{c}")
            nc.sync.dma_start(xs[:], xspk_d[c])
            epi = kpool.tile([P, DB, EC], BF, tag="epi")
            ps_sp = ps_st.tile([P, 2 * F_BLK], F32, tag="sums")
            ps_s = ps_sp[:, 0:F_BLK]
            ps_s2 = ps_sp[:, F_BLK:2 * F_BLK]
            for g in range(NGRP):
                ab_ps = k1_bcast(arow, g, "bc1")
                sq = hbpool.tile([P, DB, GE], BF, tag="sq")
                esl = epi[:, :, g * GE:(g + 1) * GE]
                abb = ab_ps[:].unsqueeze(1).to_broadcast([P, DB, GE])
                vp = proj_g(xs, g, TV0, bv0c, "vpg")
                nc.vector.tensor_tensor(esl, vp[:], abb, op=OP.mult)
                nc.vector.tensor_tensor(esl, esl, q0[:, :, g * GE:(g + 1) * GE],
                                        op=OP.add)
                nc.scalar.activation(sq[:], epi[:, :, g * GE:(g + 1) * GE], AT.Square, bias=zero_c[:])
                edge_stats(epi[:, :, g * GE:(g + 1) * GE], ps_s, g)
                edge_stats(sq[:], ps_s2, g)
            st_sb = wpool.tile([P, 2 * F_BLK], F32, tag="stsb")
            nc.vector.tensor_copy(st_sb[:], ps_sp[:])
            c0state[c] = (epi, xs, st_sb)

        def c0_back(c):
            epi, xs, st_sb = c0state.pop(c)
            ab = ln_ab(st_sb[:, 0:F_BLK], st_sb[:, F_BLK:2 * F_BLK], "ab0")
            abrow = bcast_rows(ab[:], 2 * F_BLK, "abr")
            ps_a = ps_st.tile([P, F_BLK], F32, tag="sums")
            for g in range(NGRP):
                a_ps = k1_bcast(abrow, g, "bc1")
                ln = wpool.tile([P, DB, GE], BF, tag="ln")
                nc.vector.tensor_tensor(
                    ln[:], epi[:, :, g * GE:(g + 1) * GE],
                    a_ps[:].unsqueeze(1).to_broadcast([P, DB, GE]), op=OP.mult)
                brow_g = abrow[:, F_BLK * P + g * GE:F_BLK * P + (g + 1) * GE]
                query1 = mm_group(lambda i: ln[:, i, :], w_sb["wff0t"], DB, DB,
                                  "qry", bias=bff0, brow_g=brow_g, wsum=wsff0)
                q1 = mm_group(lambda i: query1[:, i, :], w_sb["wq1t"], DB, DB,
                              "q1", bias=bq1)
                nc.sync.dma_start(q1s_d[c, :, :, g * GE:(g + 1) * GE], q1[:])
                k1p = proj_g(xs, g, TK1, bk1c, "k1p")
                prod = wpool.tile([P, DB, GE], BF, tag="prod")
                nc.vector.tensor_tensor(prod[:], q1[:], k1p[:], op=OP.mult)
                edge_stats(prod[:], ps_a[:], g)
            nc.vector.tensor_copy(alpha[:, c * F_BLK:(c + 1) * F_BLK], ps_a[:])

        for c in range(NCH + 1):
            if c < NCH:
                c0_front(c)
            if c >= 1:
                c0_back(c - 1)

        scans()

        # ===== pass C1 + head ================================================
        for c in range(NCH):
            arow = bcast_rows(attn[:, c * F_BLK:(c + 1) * F_BLK], F_BLK, "ar")
            q1l = gpool.tile([P, DB, EC], BF, tag="g1")
            nc.sync.dma_start(q1l[:], q1s_d[c])
            xs1 = gpool.tile([P, DB, EC], BF, tag="gv", name=f"xsc1{c}")
            nc.sync.dma_start(xs1[:], xspk_d[c])
            epi = kpool.tile([P, DB, EC], BF, tag="epi")
            ps_sp = ps_st.tile([P, 2 * F_BLK], F32, tag="sums")
            ps_s = ps_sp[:, 0:F_BLK]
            ps_s2 = ps_sp[:, F_BLK:2 * F_BLK]
            for g in range(NGRP):
                ab_ps = k1_bcast(arow, g, "bc1")
                sq = hbpool.tile([P, DB, GE], BF, tag="sq")
                esl = epi[:, :, g * GE:(g + 1) * GE]
                abb = ab_ps[:].unsqueeze(1).to_broadcast([P, DB, GE])
                vp = proj_g(xs1, g, TV1, bv1c, "vpg")
                nc.vector.tensor_tensor(esl, vp[:], abb, op=OP.mult)
                nc.vector.tensor_tensor(esl, esl, q1l[:, :, g * GE:(g + 1) * GE],
                                        op=OP.add)
                nc.scalar.activation(sq[:], epi[:, :, g * GE:(g + 1) * GE], AT.Square, bias=zero_c[:])
                edge_stats(epi[:, :, g * GE:(g + 1) * GE], ps_s, g)
                edge_stats(sq[:], ps_s2, g)
            ab = ln_ab(ps_s, ps_s2, "ab1")
            abrow = bcast_rows(ab[:], 2 * F_BLK, "abr")
            # t = lrelu(LN1@Wff1): built per group, kept chunk-sized
            t = kpool.tile([P, DB, EC], BF, tag="t")
            ps_sp = ps_st.tile([P, 2 * F_BLK], F32, tag="sums")
            ps_s = ps_sp[:, 0:F_BLK]
            ps_s2 = ps_sp[:, F_BLK:2 * F_BLK]
            for g in range(NGRP):
                a_ps = k1_bcast(abrow, g, "bc1")
                ln = wpool.tile([P, DB, GE], BF, tag="ln")
                nc.vector.tensor_tensor(
                    ln[:], epi[:, :, g * GE:(g + 1) * GE],
                    a_ps[:].unsqueeze(1).to_broadcast([P, DB, GE]), op=OP.mult)
                brow_g = abrow[:, F_BLK * P + g * GE:F_BLK * P + (g + 1) * GE]
                query2 = mm_group(lambda i: ln[:, i, :], w_sb["wff1t"], DB, DB,
                                  "qry", bias=bff1, brow_g=brow_g, wsum=wsff1)
                sq = hbpool.tile([P, DB, GE], BF, tag="sq")
                nc.vector.scalar_tensor_tensor(
                    t[:, :, g * GE:(g + 1) * GE], query2[:], 0.2, query2[:],
                    op0=OP.mult, op1=OP.max)
                nc.scalar.activation(sq[:], t[:, :, g * GE:(g + 1) * GE], AT.Square, bias=zero_c[:])
                edge_stats(t[:, :, g * GE:(g + 1) * GE], ps_s, g)
                edge_stats(sq[:], ps_s2, g)
            ab = ln_ab(ps_s, ps_s2, "abh")
            abrow = bcast_rows(ab[:], 2 * F_BLK, "abr")
            u = kpool.tile([P, DB, EC], BF, tag="u")
            r = kpool.tile([P, DB, EC], BF, tag="epi", name=f"r{c}")
            ps_sp = ps_st.tile([P, 2 * F_BLK], F32, tag="sums")
            ps_s = ps_sp[:, 0:F_BLK]
            ps_s2 = ps_sp[:, F_BLK:2 * F_BLK]
            for g in range(NGRP):
                a_ps = k1_bcast(abrow, g, "bc1")
                b_ps = k1_bcast(abrow[:, F_BLK * P:], g, "bc1")
                usl = u[:, :, g * GE:(g + 1) * GE]
                nc.vector.tensor_tensor(
                    usl, t[:, :, g * GE:(g + 1) * GE],
                    a_ps[:].unsqueeze(1).to_broadcast([P, DB, GE]), op=OP.mult)
                nc.vector.tensor_tensor(
                    usl, usl, b_ps[:].unsqueeze(1).to_broadcast([P, DB, GE]),
                    op=OP.add)
                for i in range(DB):
                    ug = gsl(u, i, g)
                    nc.scalar.activation(ug, ug, AT.Identity,
                                         bias=bfh[:, i:i + 1], scale=gf[:, i:i + 1])
                h1 = mm_group(lambda i: gsl(u, i, g), w_sb["w3t"], DB, HB,
                              "h1", bias=b3, act="lrelu", pool=hbpool)
                h2 = mm_group(lambda i: h1[:, i, :], w_sb["w4t"], HB, HB,
                              "h2", bias=b4, act="lrelu", pool=hbpool)
                h3 = mm_group(lambda i: h2[:, i, :], w_sb["w5t"], HB, DB,
                              "h3", bias=b5)
                sq = hbpool.tile([P, DB, GE], BF, tag="sq")
                nc.vector.tensor_tensor(r[:, :, g * GE:(g + 1) * GE], h3[:],
                                        u[:, :, g * GE:(g + 1) * GE], op=OP.add)
                nc.scalar.activation(sq[:], r[:, :, g * GE:(g + 1) * GE], AT.Square, bias=zero_c[:])
                edge_stats(r[:, :, g * GE:(g + 1) * GE], ps_s, g)
                edge_stats(sq[:], ps_s2, g)
            ab = ln_ab(ps_s, ps_s2, "abf")
            abrow = bcast_rows(ab[:], 2 * F_BLK, "abr")
            adj = rpool.tile([1, EC], F32, tag="rows", name="adj")
            for g in range(NGRP):
                a_ps = k1_bcast(abrow, g, "bc1")
                z = wpool.tile([P, DB, GE], BF, tag="ln")
                nc.vector.tensor_tensor(
                    z[:], r[:, :, g * GE:(g + 1) * GE],
                    a_ps[:].unsqueeze(1).to_broadcast([P, DB, GE]), op=OP.mult)
                brow_g = abrow[:, F_BLK * P + g * GE:F_BLK * P + (g + 1) * GE]
                ps = ps_mm.tile([1, GE], F32, tag="mmps")
                for i in range(DB):
                    nc.tensor.matmul(ps[:], lhsT=w_sb["wvt"][:, i, :],
                                     rhs=z[:, i, :], start=(i == 0), stop=False)
                nc.tensor.matmul(ps[:], lhsT=wsv[:], rhs=brow_g,
                                 start=False, stop=True)
                nc.scalar.activation(adj[:, g * GE:(g + 1) * GE], ps[:],
                                     AT.Identity, bias=bvec[:])
            nc.sync.dma_start(out_d[c * EC:(c + 1) * EC].unsqueeze(0), adj[:])

        for cm in reversed(_cms):
            cm.__exit__(None, None, None)

    nc.compile()
    return nc


# ----------------------------------------------------------------------------
# host wrapper
# ----------------------------------------------------------------------------

def prep_inputs(inputs):
    ei = np.asarray(inputs["edge_index"])
    x = np.asarray(inputs["x"], np.float32)
    g = lambda k: np.asarray(inputs[k], np.float32)
    Wq, bq, Wk, bk = g("Wq"), g("bq"), g("Wk"), g("bk")
    Wv, bv, Wff, bff = g("Wv"), g("bv"), g("Wff"), g("bff")
    ga, ba, gf, bf = g("ga"), g("ba"), g("gf"), g("bf")
    gfin, bfin = g("gfin"), g("bfin")
    W3, b3, W4, b4 = g("W3"), g("b3"), g("W4"), g("b4")
    W5, b5, Wvec, bvec = g("W5"), g("b5"), g("Wvec"), g("bvec")

    src_idx, dst_idx, m_cont, is_last, orig = pack(ei, NODES)

    Wff0p = Wff[0] * ga[0][None, :]
    bff0p = bff[0] + Wff[0] @ ba[0]
    Wff1p = Wff[1] * ga[1][None, :]
    bff1p = bff[1] + Wff[1] @ ba[1]
    Wvecp = Wvec * gfin[None, :]
    bvecp = bvec + Wvec @ bfin

    x_pad = np.zeros((NODES_P, D), np.float32)
    x_pad[:NODES] = x
    xb16 = x.astype(bf16)

    def pack_x(slot_idx):
        out = np.empty((NCH, P, DB, EC), bf16)
        for c in range(NCH):
            nodes = slot_idx[:, c * F_BLK:(c + 1) * F_BLK].T.reshape(-1)
            xg = xb16[nodes]
            out[c] = np.transpose(xg.reshape(EC, DB, P), (2, 1, 0))
        return out

    common = {
        "x_pad": x_pad,
        "wtab": np.ascontiguousarray(np.concatenate(
            [Wq[0].T, Wk[0].T, Wv[0].T, Wk[1].T, Wv[1].T], axis=1)),
        "wq1t": np.ascontiguousarray(Wq[1].T),
        "wff0t": np.ascontiguousarray(Wff0p.T),
        "wff1t": np.ascontiguousarray(Wff1p.T),
        "w3t": np.ascontiguousarray(W3.T),
        "w4t": np.ascontiguousarray(W4.T),
        "w5t": np.ascontiguousarray(W5.T),
        "wvt": np.ascontiguousarray(Wvecp.T),
        "bq1_t": feature_blocks(bq[1]),
        "bff0_t": feature_blocks(bff0p),
        "bff1_t": feature_blocks(bff1p),
        "b3_t": feature_blocks(b3),
        "b4_t": feature_blocks(b4),
        "b5_t": feature_blocks(b5),
        "gf_t": feature_blocks(gf[0]),
        "bf_t": feature_blocks(bf[0]),
        "bvec_t": bvecp.reshape(1, 1).astype(np.float32),
        "btab_r": np.broadcast_to(
            np.concatenate([bq[0], bk[0], bv[0], bk[1], bv[1]]), (P, 5 * D)).copy(),
        "wsff0": Wff0p.T.sum(0, keepdims=True).astype(np.float32),
        "wsff1": Wff1p.T.sum(0, keepdims=True).astype(np.float32),
        "wsv": Wvecp.T.sum(0, keepdims=True).astype(np.float32),
        "bq0_t": feature_blocks(bq[0]),
        "bk0_t": feature_blocks(bk[0]),
        "bv0_t": feature_blocks(bv[0]),
        "bk1_t": feature_blocks(bk[1]),
        "bv1_t": feature_blocks(bv[1]),
    }
    in_maps = []
    for c in range(N_CORES):
        st_add = np.where(m_cont[c] > 0, 0.0, NEG).astype(np.float32)
        m = dict(common)
        m.update({
            "gsrc": gather_idx_layout(src_idx[c]),
            "gdst": gather_idx_layout(dst_idx[c]),
            "xspk": pack_x(src_idx[c]),
            "xdpk": pack_x(dst_idx[c]),
            "st_add": st_add,
            "m_cont": m_cont[c],
            "notlast": (1.0 - is_last[c]).astype(np.float32),
            "is_last": is_last[c],
        })
        in_maps.append(m)
    return in_maps, orig


def unshard(results, orig, E):
    out = np.zeros(E, np.float32)
    for c in range(N_CORES):
        core_out = np.asarray(results[c]["out"]).reshape(E_CAP)
        vals = core_out.reshape(NCH, F_BLK, P)       # [chunk, j, p]
        vals = np.transpose(vals, (2, 0, 1)).reshape(P, F_CAP)
        o = orig[c]
        m = o >= 0
        out[o[m]] = vals[m]
    return out


def kernel(**inputs) -> np.ndarray:
    in_maps, orig = prep_inputs(inputs)
    nc = build_nc()
    res = run_bass_kernel_spmd(nc, in_maps, core_ids=list(range(N_CORES)))
    return unshard(res.results, orig, int(np.asarray(inputs["edge_index"]).shape[1]))

